# revision 2
# baseline (speedup 1.0000x reference)
"""Trainium2 Bass kernel for nn_ChiralEmbeddingModel — v2.

vs v1 baseline:
 - host pre-transposes activations to feature-major HBM layout: no on-chip
   input transposes
 - gate path (inv @ g_w1 -> silu -> @ g_w2 -> sigmoid) in fp8e4 DoubleRow
   matmuls (0.5 cy/row, 2 k-tiles packed: 4x vs fp16)
 - eq path fp16; cross/dot chain on DVE (one PSUM operand per op, x0
   staged to SBUF via ACT)
 - chi/tanh transposed to atom-major via regular matmuls against
   [identity | hf-masks]: transpose AND LayerNorm K-sums in one pass
 - Newton rsqrt + LN/gate scale fixups on GPSIMD; outF on GPSIMD
 - 4-deep software pipelining so every cross-engine/psum dependency has
   at least one full tile-iteration of slack
 - outputs fp16 in (t, p, s, k) layout; host reorders + casts to fp32
"""
import os
import sys

sys.path.insert(0, '/opt/trn_rl_repo')

import numpy as np

import concourse.bass as bass
import concourse.bacc as bacc
import concourse.mybir as mybir
import concourse.tile as tile
from concourse.bass_utils import run_bass_kernel_spmd

N, INV, M, K, H = 131072, 256, 256, 64, 512
N_CORES = 8
N_CORE = N // N_CORES          # 16384 atoms per core
T = 512                        # atoms per tile
Th = T // 2
NT = N_CORE // T               # 32 tiles
LN_EPS = 1e-5
GW_SCALE = 16.0                # gate weights stored x16 in fp8; undone in ACT
F16 = mybir.dt.float16
F32 = mybir.dt.float32
F8 = mybir.dt.float8e4
I32 = mybir.dt.int32
AF = mybir.ActivationFunctionType
ALU = mybir.AluOpType
DR = mybir.MatmulPerfMode.DoubleRow

LAST_RESULT = None
_NC_CACHE = None


def _ap_view(t, offset_elems, dims):
    """Raw AP on tile t's tensor: partition dim kept, custom free dims."""
    return bass.AP(tensor=t.tensor, offset=t.offset + offset_elems,
                   ap=[list(t.ap[0])] + [list(d) for d in dims])


def build_nc():
    nc = bacc.Bacc("TRN2", target_bir_lowering=False)
    eqT = nc.dram_tensor("eqT", [128, NT, 6, T], F16, kind="ExternalInput")
    invT = nc.dram_tensor("invT", [128, NT, 2, 2, T], F8, kind="ExternalInput")
    wcat = nc.dram_tensor("wcat", [128, 2, 192], F16, kind="ExternalInput")
    gw1 = nc.dram_tensor("gw1", [128, 2, 2, 4, 128], F8, kind="ExternalInput")
    gb1 = nc.dram_tensor("gb1", [128, 4], F32, kind="ExternalInput")
    gw2 = nc.dram_tensor("gw2", [128, 4, K], F16, kind="ExternalInput")
    gb2 = nc.dram_tensor("gb2", [64, 1], F32, kind="ExternalInput")
    idm = nc.dram_tensor("idm", [128, 136], F16, kind="ExternalInput")
    out = nc.dram_tensor("out", [NT, 128, 4, K], F16, kind="ExternalOutput")

    with tile.TileContext(nc) as tc:
        with (
            tc.tile_pool(name="const", bufs=1) as const,
            tc.tile_pool(name="inp", bufs=7) as inp,
            tc.tile_pool(name="act", bufs=5) as act,
            tc.tile_pool(name="ps", bufs=1, space="PSUM") as ps,
        ):
            idm_sb = const.tile([128, 136], F16)
            nc.sync.dma_start(out=idm_sb, in_=idm[:, :])
            wcat_sb = const.tile([128, 2, 192], F16)
            nc.sync.dma_start(out=wcat_sb, in_=wcat[:, :, :])
            gw1_sb = const.tile([128, 2, 2, 4, 128], F8)
            nc.sync.dma_start(out=gw1_sb, in_=gw1[:, :, :, :, :])
            gb1_sb = const.tile([128, 4], F32)
            nc.sync.dma_start(out=gb1_sb, in_=gb1[:, :])
            gw2_sb = const.tile([128, 4, K], F16)
            nc.sync.dma_start(out=gw2_sb, in_=gw2[:, :, :])
            gb2_sb = const.tile([64, 1], F32)
            nc.sync.dma_start(out=gb2_sb, in_=gb2[:, :])
            # sq-with-eps tile: col 64 holds 64*eps; cols 0:64 rewritten
            sqe = const.tile([128, 4, 65], F16)
            nc.sync.dma_start(
                out=_ap_view(sqe, 64, [[65, 4], [1, 1]]),
                in_=idm[:, 130:134].rearrange("p (f x) -> p f x", x=1))

            def prefetch(t):
                eq_sb = inp.tile([128, 6, T], F16)
                nc.sync.dma_start(out=eq_sb, in_=eqT[:, t])
                inv_sb = inp.tile([128, 2, 2, T], F8)
                nc.sync.dma_start(out=inv_sb, in_=invT[:, t])
                return {"eq_sb": eq_sb, "inv_sb": inv_sb}

            def xmm_c(eq_sb, dst, w, sl, c):
                for hf in range(2):
                    for hh in range(2):
                        nc.tensor.matmul(
                            dst[64 * hf:64 * (hf + 1), sl + c, :],
                            wcat_sb[:, hh, 64 * w:64 * (w + 1)],
                            eq_sb[:, c * 2 + hh, Th * hf:Th * (hf + 1)],
                            start=(hh == 0), stop=(hh == 1),
                            tile_position=(0, 64 * hf))

            def head(t, st):
                eq_sb, inv_sb = st["eq_sb"], st["inv_sb"]

                # gate layer 1: residual-compensated fp8 DR
                # z1 ~= xhi@whi + xhi@wlo + xlo@whi  (x = inv, w = gw1*16)
                g1s = act.tile([128, 4, T], F16)
                st["g1s"] = g1s
                for g in range(4):
                    g1p = ps.tile([128, T], F32, tag="g1", bufs=2)
                    for i, (hl, xl) in enumerate(((0, 0), (1, 0), (0, 1))):
                        nc.tensor.matmul(g1p, gw1_sb[:, hl, :, g, :],
                                         inv_sb[:, xl, :, :],
                                         start=(i == 0), stop=(i == 2),
                                         perf_mode=DR)
                    nc.scalar.activation(out=g1s[:, g, :], in_=g1p,
                                         func=AF.Silu, scale=1.0 / GW_SCALE,
                                         bias=gb1_sb[:, g:g + 1])

                # eq GEMMs: x0/y1 share one psum tile (slots 0:3 / 3:6);
                # ACT stages the whole tile to SBUF fp16 (hw allows only one
                # PSUM operand per vector op; SBUF fp16 also gives 2x DVE)
                xy = ps.tile([128, 6, Th], F32, tag="xy")
                xyS = act.tile([128, 6, Th], F16)
                for c in range(3):
                    xmm_c(eq_sb, xy, 0, 0, c)
                nc.scalar.copy(out=xyS[:, 0:3, :], in_=xy[:, 0:3, :])
                for c in range(3):
                    xmm_c(eq_sb, xy, 1, 3, c)
                nc.scalar.copy(out=xyS[:, 3:6, :], in_=xy[:, 3:6, :])

                # cross products:
                # P0=x0_1*y1_2, P1=x0_2*y1_1 | P2=x0_2*y1_0, P3=x0_0*y1_2
                # P4=x0_0*y1_1, P5=x0_1*y1_0
                Pall = act.tile([128, 6, Th], F16)
                nc.vector.tensor_tensor(
                    out=Pall[:, 0:2, :], in0=xyS[:, 1:3, :],
                    in1=_ap_view(xyS, 5 * Th, [[-Th, 2], [1, Th]]),
                    op=ALU.mult)
                nc.vector.tensor_tensor(
                    out=Pall[:, 2:4, :],
                    in0=_ap_view(xyS, 2 * Th, [[-2 * Th, 2], [1, Th]]),
                    in1=_ap_view(xyS, 3 * Th, [[2 * Th, 2], [1, Th]]),
                    op=ALU.mult)
                nc.vector.tensor_tensor(
                    out=Pall[:, 4:6, :], in0=xyS[:, 0:2, :],
                    in1=_ap_view(xyS, 4 * Th, [[-Th, 2], [1, Th]]),
                    op=ALU.mult)
                crossall = act.tile([128, 3, Th], F16)
                pv = Pall.rearrange("p (c two) f -> p c two f", two=2)
                nc.vector.tensor_tensor(out=crossall, in0=pv[:, :, 0, :],
                                        in1=pv[:, :, 1, :], op=ALU.subtract)
                st["crossall"] = crossall

            def head2(st):
                """y2 GEMM + dot + gate layer 2, one tile late: psum reuse
                and the pd chain get a full iteration of slack."""
                eq_sb, crossall = st["eq_sb"], st["crossall"]
                y2p = ps.tile([128, 3, Th], F32, tag="y2")
                for c in range(3):
                    xmm_c(eq_sb, y2p, 2, 0, c)

                # dot: chi = sum_c cross_c * y2_c  (y2 read from PSUM)
                pd = act.tile([128, 3, Th], F16)
                nc.vector.tensor_tensor(out=pd, in0=crossall, in1=y2p,
                                        op=ALU.mult)
                chiF = act.tile([128, Th], F16)
                nc.vector.tensor_tensor(out=chiF, in0=pd[:, 0, :],
                                        in1=pd[:, 1, :], op=ALU.add)
                nc.vector.tensor_tensor(out=chiF, in0=chiF, in1=pd[:, 2, :],
                                        op=ALU.add)
                st["chiF"] = chiF

                # gate layer 2 (fp16, accumulate over 4 H-blocks) + tanh
                g2p = ps.tile([64, T], F32, tag="gt")
                for j in range(4):
                    nc.tensor.matmul(g2p, gw2_sb[:, j, :],
                                     st["g1s"][:, j, :],
                                     start=(j == 0), stop=(j == 3))
                tanhS = act.tile([64, T], F16)
                nc.scalar.activation(out=tanhS, in_=g2p, func=AF.Tanh,
                                     scale=0.5, bias=gb2_sb[:, 0:1])
                st["tanhS"] = tanhS

            def trans(st):
                """chi/tanh -> atom-major fp16 psum (own 1-bank tag).
                tp slots: [0:4) chi as [r, hf, k]; [4:8) tanh blocks q."""
                chiF, tanhS = st["chiF"], st["tanhS"]
                tp = ps.tile([128, 8, 64], F16, tag="gt")
                for r in range(2):
                    nc.tensor.transpose(
                        tp[:, 2 * r:2 * r + 2, :],
                        chiF[:, r * 128:(r + 1) * 128], idm_sb[:, 0:128])
                for q in range(4):
                    s = (q % 2) * 2 + q // 2
                    nc.tensor.transpose(
                        tp[:, 4 + s, :],
                        tanhS[:, q * 128:(q + 1) * 128],
                        idm_sb[0:64, 0:64])
                st["tp"] = tp

            def tail_stats(st):
                """LN stats + mean/rstd/gate-scale tail (DVE + GPSIMD)."""
                tp = st["tp"]
                # stage to SBUF (psum one-input rule for sq; cheap 2x copy)
                tcS = act.tile([128, 8, 64], F16)
                nc.vector.tensor_copy(out=tcS, in_=tp)
                chg = tcS[:, 0:4, :]                   # [128, s=(r,hf), k]
                S1 = act.tile([128, 4], F32)
                nc.vector.tensor_reduce(out=S1, in_=chg,
                                        axis=mybir.AxisListType.X, op=ALU.add)
                nc.vector.tensor_tensor(out=sqe[:, :, 0:64], in0=chg,
                                        in1=chg, op=ALU.mult)
                S2 = act.tile([128, 4], F32)
                nc.vector.tensor_reduce(out=S2, in_=sqe,
                                        axis=mybir.AxisListType.X, op=ALU.add)

                # veps = S2 - S1^2/64 (+64eps already inside S2)
                t1 = act.tile([128, 4], F32)
                nc.gpsimd.tensor_tensor(out=t1, in0=S1, in1=S1, op=ALU.mult)
                veps = act.tile([128, 4], F32)
                nc.vector.scalar_tensor_tensor(out=veps, in0=t1,
                                               scalar=-1.0 / 64.0, in1=S2,
                                               op0=ALU.mult, op1=ALU.add)
                m = act.tile([128, 4], F32)
                nc.vector.tensor_scalar(out=m, in0=S1, scalar1=1.0 / 64.0,
                                        scalar2=None, op0=ALU.mult)
                # Newton rsqrt (1 iter), x4 folded: rstd4 = 4/sqrt(veps)
                # (true rstd = 8/sqrt(veps) = 2*rstd4)
                ii = act.tile([128, 4], I32)
                nc.vector.tensor_scalar(out=ii, in0=veps.bitcast(I32),
                                        scalar1=1, scalar2=-1,
                                        op0=ALU.arith_shift_right,
                                        op1=ALU.bitwise_xor)
                r0 = act.tile([128, 4], F32)
                nc.vector.tensor_scalar(out=r0.bitcast(I32), in0=ii,
                                        scalar1=0x5f3759df + 1, scalar2=None,
                                        op0=ALU.add)
                tN = act.tile([128, 4], F32)
                nc.gpsimd.tensor_tensor(out=tN, in0=r0, in1=r0, op=ALU.mult)
                nc.gpsimd.tensor_tensor(out=tN, in0=tN, in1=veps,
                                        op=ALU.mult)
                nc.vector.tensor_scalar(out=tN, in0=tN, scalar1=-0.5,
                                        scalar2=1.5, op0=ALU.mult,
                                        op1=ALU.add)
                r1 = act.tile([128, 4], F32)
                nc.gpsimd.tensor_tensor(out=r1, in0=r0, in1=tN, op=ALU.mult)
                # second Newton iteration (x4 folded into the affine)
                tM = act.tile([128, 4], F32)
                nc.gpsimd.tensor_tensor(out=tM, in0=r1, in1=r1, op=ALU.mult)
                nc.gpsimd.tensor_tensor(out=tM, in0=tM, in1=veps,
                                        op=ALU.mult)
                nc.vector.tensor_scalar(out=tM, in0=tM, scalar1=-2.0,
                                        scalar2=6.0, op0=ALU.mult,
                                        op1=ALU.add)
                rstd4 = act.tile([128, 4], F32)
                nc.gpsimd.tensor_tensor(out=rstd4, in0=r1, in1=tM,
                                        op=ALU.mult)

                # gs = tanh*(0.5*rstd) + 0.5*rstd, via stride-0 broadcast of
                # rstd4 (== 0.5*rstd_true) along k; Pool tensor_tensor ops
                Rb = _ap_view(rstd4, 0, [[1, 4], [0, 64]])
                Mb = _ap_view(m, 0, [[1, 4], [0, 64]])
                gs = act.tile([128, 4, 64], F16)
                nc.gpsimd.tensor_tensor(out=gs, in0=tcS[:, 4:8, :], in1=Rb,
                                        op=ALU.mult)
                nc.gpsimd.tensor_tensor(out=gs, in0=gs, in1=Rb, op=ALU.add)
                olN = act.tile([128, 4, 64], F16)
                nc.gpsimd.tensor_tensor(out=olN, in0=tcS[:, 0:4, :], in1=Mb,
                                        op=ALU.subtract)
                st["gs"], st["olN"] = gs, olN

            def tail_out(st, t):
                """outF + store, three tiles late."""
                outF = act.tile([128, 4, K], F16)
                nc.gpsimd.tensor_tensor(out=outF, in0=st["olN"],
                                        in1=st["gs"], op=ALU.mult)
                nc.sync.dma_start(out=out[t], in_=outF)

            sts = [prefetch(0), prefetch(1)]
            for t in range(NT + 4):
                # prefetch first: the out-DMA below blocks the SP queue on
                # outF, and inputs must stay well ahead of the xy matmuls
                if 2 <= t + 2 < NT:
                    sts.append(prefetch(t + 2))
                if 0 <= t - 3 < NT:
                    trans(sts[t - 3])
                    tail_stats(sts[t - 3])
                if 0 <= t - 2 < NT:
                    head2(sts[t - 2])
                if t - 4 >= 0:
                    tail_out(sts[t - 4], t - 4)
                if t < NT:
                    head(t, sts[t])
    nc.compile()
    return nc


def _prep_weights(mean_inv, std_inv, rms_gamma, W0, W1, W2, w_cross, w_dot,
                  g_w1, g_b1, g_w2, g_b2):
    import ml_dtypes
    f8 = ml_dtypes.float8_e4m3fn
    g = (rms_gamma.astype(np.float64) / np.sqrt(M))
    W0s = W0.astype(np.float64) * g[:, None]
    Wy1 = (W1.astype(np.float64) * g[:, None]) @ (
        w_cross.T.astype(np.float64) / np.sqrt(2.0 * K))
    Wy2 = (W2.astype(np.float64) * g[:, None]) @ (
        w_dot.T.astype(np.float64) / np.sqrt(3.0 * K))
    Wcat = np.concatenate([W0s, Wy1, Wy2], axis=1)          # [256, 192]
    wcat = Wcat.reshape(2, 128, 192).transpose(1, 0, 2).astype(np.float16)

    inv_std = 1.0 / std_inv.astype(np.float64)
    GW1 = g_w1.astype(np.float64) * inv_std[:, None] * GW_SCALE  # [256, 512]
    w1r = GW1.reshape(2, 128, 4, 128).transpose(1, 0, 2, 3)
    w1hi = w1r.astype(f8)
    w1lo = (w1r - w1hi.astype(np.float64)).astype(f8)
    gw1 = np.stack([w1hi, w1lo], axis=1)       # [128, hl, kt, g, h]
    GB1 = g_b1.astype(np.float64) - (mean_inv.astype(np.float64)
                                     * inv_std) @ g_w1.astype(np.float64)
    gb1 = GB1.reshape(4, 128).T.astype(np.float32).copy()
    gw2 = g_w2.astype(np.float64).reshape(4, 128, K).transpose(
        1, 0, 2).astype(np.float16)            # [128, hh, k]
    gb2 = (0.5 * g_b2.astype(np.float64)).reshape(K, 1).astype(np.float32)
    idm = np.zeros((128, 136), np.float16)
    idm[:, :128] = np.eye(128, dtype=np.float16)
    idm[0:64, 128] = 1.0          # S1 mask for hf=0
    idm[64:128, 129] = 1.0        # S1 mask for hf=1
    idm[:, 130:134] = 64.0 * LN_EPS
    return dict(wcat=np.ascontiguousarray(wcat), gw1=np.ascontiguousarray(gw1),
                gb1=gb1, gw2=np.ascontiguousarray(gw2), gb2=gb2, idm=idm)


def _prep_acts(emb):
    """emb [N, 1024] -> per-core (eqT fp16, invT fp8) feature-major."""
    import ml_dtypes
    f8 = ml_dtypes.float8_e4m3fn
    res = []
    for c in range(N_CORES):
        A = np.asarray(emb[c * N_CORE:(c + 1) * N_CORE])
        E = A[:, INV:].reshape(NT, T, 2, 128, 3).transpose(3, 0, 4, 2, 1)
        eqT = np.ascontiguousarray(E.reshape(128, NT, 6, T).astype(np.float16))
        I = A[:, :INV].reshape(NT, T, 2, 128).transpose(3, 0, 2, 1)
        ihi = I.astype(f8)
        ilo = (I - ihi.astype(np.float64)).astype(f8)
        invT = np.ascontiguousarray(
            np.stack([ihi, ilo], axis=2))      # [128, t, hl, hh, a]
        res.append((eqT, invT))
    return res


def kernel(atomic_embeddings, mean_inv, std_inv, rms_gamma, W0, W1, W2,
           w_cross, w_dot, ln_w, ln_b, g_w1, g_b1, g_w2, g_b2):
    global _NC_CACHE, LAST_RESULT
    assert np.allclose(np.asarray(ln_w), 1.0) and np.allclose(np.asarray(ln_b), 0.0), \
        "kernel specialized for ln_w=1, ln_b=0"
    weights = _prep_weights(np.asarray(mean_inv), np.asarray(std_inv),
                            np.asarray(rms_gamma), np.asarray(W0),
                            np.asarray(W1), np.asarray(W2),
                            np.asarray(w_cross), np.asarray(w_dot),
                            np.asarray(g_w1), np.asarray(g_b1),
                            np.asarray(g_w2), np.asarray(g_b2))
    acts = _prep_acts(np.asarray(atomic_embeddings))
    if _NC_CACHE is None:
        _NC_CACHE = build_nc()
    nc = _NC_CACHE
    in_maps = []
    for c in range(N_CORES):
        mm = dict(weights)
        mm["eqT"], mm["invT"] = acts[c]
        in_maps.append(mm)
    trace = bool(int(os.environ.get("CHIRAL_TRACE", "0")))
    try:
        from antenv import axon_hooks  # noqa: F401
    except ImportError:
        os.environ["BASS_NEVER_TRACE"] = "1"
        trace = False
    res = run_bass_kernel_spmd(nc, in_maps, core_ids=list(range(N_CORES)),
                               trace=trace)
    LAST_RESULT = res
    outs = []
    for c in range(N_CORES):
        o = res.results[c]["out"]                    # [NT, 128, 4, 64] fp16
        o = o.reshape(NT, 128, 2, 2, K).transpose(0, 3, 2, 1, 4)
        outs.append(o.reshape(N_CORE, K).astype(np.float32))
    return np.concatenate(outs, axis=0)


# revision 3
# speedup vs baseline: 1.0033x; 1.0033x over previous
"""Trainium2 Bass kernel for nn_ChiralEmbeddingModel — v2.

vs v1 baseline:
 - host pre-transposes activations to feature-major HBM layout: no on-chip
   input transposes
 - gate path (inv @ g_w1 -> silu -> @ g_w2 -> sigmoid) in fp8e4 DoubleRow
   matmuls (0.5 cy/row, 2 k-tiles packed: 4x vs fp16)
 - eq path fp16; cross/dot chain on DVE (one PSUM operand per op, x0
   staged to SBUF via ACT)
 - chi/tanh transposed to atom-major via regular matmuls against
   [identity | hf-masks]: transpose AND LayerNorm K-sums in one pass
 - Newton rsqrt + LN/gate scale fixups on GPSIMD; outF on GPSIMD
 - 4-deep software pipelining so every cross-engine/psum dependency has
   at least one full tile-iteration of slack
 - outputs fp16 in (t, p, s, k) layout; host reorders + casts to fp32
"""
import os
import sys

sys.path.insert(0, '/opt/trn_rl_repo')

import numpy as np

import concourse.bass as bass
import concourse.bacc as bacc
import concourse.mybir as mybir
import concourse.tile as tile
from concourse.bass_utils import run_bass_kernel_spmd

N, INV, M, K, H = 131072, 256, 256, 64, 512
N_CORES = 8
N_CORE = N // N_CORES          # 16384 atoms per core
T = 512                        # atoms per tile
Th = T // 2
NT = N_CORE // T               # 32 tiles
LN_EPS = 1e-5
GW_SCALE = 16.0                # gate weights stored x16 in fp8; undone in ACT
F16 = mybir.dt.float16
F32 = mybir.dt.float32
F8 = mybir.dt.float8e4
I32 = mybir.dt.int32
AF = mybir.ActivationFunctionType
ALU = mybir.AluOpType
DR = mybir.MatmulPerfMode.DoubleRow

LAST_RESULT = None
_NC_CACHE = None


def _ap_view(t, offset_elems, dims):
    """Raw AP on tile t's tensor: partition dim kept, custom free dims."""
    return bass.AP(tensor=t.tensor, offset=t.offset + offset_elems,
                   ap=[list(t.ap[0])] + [list(d) for d in dims])


def build_nc():
    nc = bacc.Bacc("TRN2", target_bir_lowering=False)
    eqT = nc.dram_tensor("eqT", [128, NT, 6, T], F16, kind="ExternalInput")
    invT = nc.dram_tensor("invT", [128, NT, 2, 2, T], F8, kind="ExternalInput")
    wcat = nc.dram_tensor("wcat", [128, 2, 192], F16, kind="ExternalInput")
    gw1 = nc.dram_tensor("gw1", [128, 2, 2, 4, 128], F8, kind="ExternalInput")
    gb1 = nc.dram_tensor("gb1", [128, 4], F32, kind="ExternalInput")
    gw2 = nc.dram_tensor("gw2", [128, 4, K], F16, kind="ExternalInput")
    gb2 = nc.dram_tensor("gb2", [64, 1], F32, kind="ExternalInput")
    idm = nc.dram_tensor("idm", [128, 136], F16, kind="ExternalInput")
    out = nc.dram_tensor("out", [NT, 128, 4, K], F16, kind="ExternalOutput")

    with tile.TileContext(nc) as tc:
        with (
            tc.tile_pool(name="const", bufs=1) as const,
            tc.tile_pool(name="inp", bufs=7) as inp,
            tc.tile_pool(name="act", bufs=5) as act,
            tc.tile_pool(name="ps", bufs=1, space="PSUM") as ps,
        ):
            # startup DMA order = first-use order: gate-1 inputs first so
            # the PE starts ~3us in instead of ~10us (SP queue + shared
            # HWDGE serialize these)
            inv0 = const.tile([128, 2, 2, T], F8)
            nc.sync.dma_start(out=inv0, in_=invT[:, 0])
            gw1_sb = const.tile([128, 2, 2, 4, 128], F8)
            nc.sync.dma_start(out=gw1_sb, in_=gw1[:, :, :, :, :])
            gb1_sb = const.tile([128, 4], F32)
            nc.sync.dma_start(out=gb1_sb, in_=gb1[:, :])
            eq0 = const.tile([128, 6, T], F16)
            nc.sync.dma_start(out=eq0, in_=eqT[:, 0])
            wcat_sb = const.tile([128, 2, 192], F16)
            nc.sync.dma_start(out=wcat_sb, in_=wcat[:, :, :])
            inv1 = const.tile([128, 2, 2, T], F8)
            nc.sync.dma_start(out=inv1, in_=invT[:, 1])
            eq1 = const.tile([128, 6, T], F16)
            nc.sync.dma_start(out=eq1, in_=eqT[:, 1])
            gw2_sb = const.tile([128, 4, K], F16)
            nc.sync.dma_start(out=gw2_sb, in_=gw2[:, :, :])
            gb2_sb = const.tile([64, 1], F32)
            nc.sync.dma_start(out=gb2_sb, in_=gb2[:, :])
            idm_sb = const.tile([128, 136], F16)
            nc.sync.dma_start(out=idm_sb, in_=idm[:, :])
            # sq-with-eps tile: col 64 holds 64*eps; cols 0:64 rewritten
            sqe = const.tile([128, 4, 65], F16)
            nc.sync.dma_start(
                out=_ap_view(sqe, 64, [[65, 4], [1, 1]]),
                in_=idm[:, 130:134].rearrange("p (f x) -> p f x", x=1))

            def prefetch(t):
                eq_sb = inp.tile([128, 6, T], F16)
                nc.sync.dma_start(out=eq_sb, in_=eqT[:, t])
                inv_sb = inp.tile([128, 2, 2, T], F8)
                nc.sync.dma_start(out=inv_sb, in_=invT[:, t])
                return {"eq_sb": eq_sb, "inv_sb": inv_sb}

            def xmm_c(eq_sb, dst, w, sl, c):
                for hf in range(2):
                    for hh in range(2):
                        nc.tensor.matmul(
                            dst[64 * hf:64 * (hf + 1), sl + c, :],
                            wcat_sb[:, hh, 64 * w:64 * (w + 1)],
                            eq_sb[:, c * 2 + hh, Th * hf:Th * (hf + 1)],
                            start=(hh == 0), stop=(hh == 1),
                            tile_position=(0, 64 * hf))

            def head(t, st):
                eq_sb, inv_sb = st["eq_sb"], st["inv_sb"]

                # gate layer 1: residual-compensated fp8 DR
                # z1 ~= xhi@whi + xhi@wlo + xlo@whi  (x = inv, w = gw1*16)
                g1s = act.tile([128, 4, T], F16)
                st["g1s"] = g1s
                for g in range(4):
                    g1p = ps.tile([128, T], F32, tag="g1", bufs=2)
                    for i, (hl, xl) in enumerate(((0, 0), (1, 0), (0, 1))):
                        nc.tensor.matmul(g1p, gw1_sb[:, hl, :, g, :],
                                         inv_sb[:, xl, :, :],
                                         start=(i == 0), stop=(i == 2),
                                         perf_mode=DR)
                    nc.scalar.activation(out=g1s[:, g, :], in_=g1p,
                                         func=AF.Silu, scale=1.0 / GW_SCALE,
                                         bias=gb1_sb[:, g:g + 1])

                # eq GEMMs: x0/y1 share one psum tile (slots 0:3 / 3:6);
                # ACT stages the whole tile to SBUF fp16 (hw allows only one
                # PSUM operand per vector op; SBUF fp16 also gives 2x DVE)
                xy = ps.tile([128, 6, Th], F32, tag="xy")
                xyS = act.tile([128, 6, Th], F16)
                for c in range(3):
                    xmm_c(eq_sb, xy, 0, 0, c)
                nc.scalar.copy(out=xyS[:, 0:3, :], in_=xy[:, 0:3, :])
                for c in range(3):
                    xmm_c(eq_sb, xy, 1, 3, c)
                nc.scalar.copy(out=xyS[:, 3:6, :], in_=xy[:, 3:6, :])

                # cross products:
                # P0=x0_1*y1_2, P1=x0_2*y1_1 | P2=x0_2*y1_0, P3=x0_0*y1_2
                # P4=x0_0*y1_1, P5=x0_1*y1_0
                Pall = act.tile([128, 6, Th], F16)
                nc.vector.tensor_tensor(
                    out=Pall[:, 0:2, :], in0=xyS[:, 1:3, :],
                    in1=_ap_view(xyS, 5 * Th, [[-Th, 2], [1, Th]]),
                    op=ALU.mult)
                nc.vector.tensor_tensor(
                    out=Pall[:, 2:4, :],
                    in0=_ap_view(xyS, 2 * Th, [[-2 * Th, 2], [1, Th]]),
                    in1=_ap_view(xyS, 3 * Th, [[2 * Th, 2], [1, Th]]),
                    op=ALU.mult)
                nc.vector.tensor_tensor(
                    out=Pall[:, 4:6, :], in0=xyS[:, 0:2, :],
                    in1=_ap_view(xyS, 4 * Th, [[-Th, 2], [1, Th]]),
                    op=ALU.mult)
                crossall = act.tile([128, 3, Th], F16)
                pv = Pall.rearrange("p (c two) f -> p c two f", two=2)
                nc.vector.tensor_tensor(out=crossall, in0=pv[:, :, 0, :],
                                        in1=pv[:, :, 1, :], op=ALU.subtract)
                st["crossall"] = crossall

            def head2(st):
                """y2 GEMM + dot + gate layer 2, one tile late: psum reuse
                and the pd chain get a full iteration of slack."""
                eq_sb, crossall = st["eq_sb"], st["crossall"]
                y2p = ps.tile([128, 3, Th], F32, tag="y2")
                for c in range(3):
                    xmm_c(eq_sb, y2p, 2, 0, c)

                # dot: chi = sum_c cross_c * y2_c  (y2 read from PSUM)
                pd = act.tile([128, 3, Th], F16)
                nc.vector.tensor_tensor(out=pd, in0=crossall, in1=y2p,
                                        op=ALU.mult)
                chiF = act.tile([128, Th], F16)
                nc.vector.tensor_tensor(out=chiF, in0=pd[:, 0, :],
                                        in1=pd[:, 1, :], op=ALU.add)
                nc.vector.tensor_tensor(out=chiF, in0=chiF, in1=pd[:, 2, :],
                                        op=ALU.add)
                st["chiF"] = chiF

                # gate layer 2 (fp16, accumulate over 4 H-blocks) + tanh
                g2p = ps.tile([64, T], F32, tag="gt")
                for j in range(4):
                    nc.tensor.matmul(g2p, gw2_sb[:, j, :],
                                     st["g1s"][:, j, :],
                                     start=(j == 0), stop=(j == 3))
                tanhS = act.tile([64, T], F16)
                nc.scalar.activation(out=tanhS, in_=g2p, func=AF.Tanh,
                                     scale=0.5, bias=gb2_sb[:, 0:1])
                st["tanhS"] = tanhS

            def trans(st):
                """chi/tanh -> atom-major fp16 psum (own 1-bank tag).
                tp slots: [0:4) chi as [r, hf, k]; [4:8) tanh blocks q."""
                chiF, tanhS = st["chiF"], st["tanhS"]
                tp = ps.tile([128, 8, 64], F16, tag="gt")
                for r in range(2):
                    nc.tensor.transpose(
                        tp[:, 2 * r:2 * r + 2, :],
                        chiF[:, r * 128:(r + 1) * 128], idm_sb[:, 0:128])
                for q in range(4):
                    s = (q % 2) * 2 + q // 2
                    nc.tensor.transpose(
                        tp[:, 4 + s, :],
                        tanhS[:, q * 128:(q + 1) * 128],
                        idm_sb[0:64, 0:64])
                st["tp"] = tp

            def tail_stats(st):
                """LN stats + mean/rstd/gate-scale tail (DVE + GPSIMD)."""
                tp = st["tp"]
                # stage to SBUF (psum one-input rule for sq; cheap 2x copy)
                tcS = act.tile([128, 8, 64], F16)
                nc.vector.tensor_copy(out=tcS, in_=tp)
                chg = tcS[:, 0:4, :]                   # [128, s=(r,hf), k]
                S1 = act.tile([128, 4], F32)
                nc.vector.tensor_reduce(out=S1, in_=chg,
                                        axis=mybir.AxisListType.X, op=ALU.add)
                nc.vector.tensor_tensor(out=sqe[:, :, 0:64], in0=chg,
                                        in1=chg, op=ALU.mult)
                S2 = act.tile([128, 4], F32)
                nc.vector.tensor_reduce(out=S2, in_=sqe,
                                        axis=mybir.AxisListType.X, op=ALU.add)

                # veps = S2 - S1^2/64 (+64eps already inside S2)
                t1 = act.tile([128, 4], F32)
                nc.gpsimd.tensor_tensor(out=t1, in0=S1, in1=S1, op=ALU.mult)
                veps = act.tile([128, 4], F32)
                nc.vector.scalar_tensor_tensor(out=veps, in0=t1,
                                               scalar=-1.0 / 64.0, in1=S2,
                                               op0=ALU.mult, op1=ALU.add)
                m = act.tile([128, 4], F32)
                nc.vector.tensor_scalar(out=m, in0=S1, scalar1=1.0 / 64.0,
                                        scalar2=None, op0=ALU.mult)
                # Newton rsqrt (1 iter), x4 folded: rstd4 = 4/sqrt(veps)
                # (true rstd = 8/sqrt(veps) = 2*rstd4)
                ii = act.tile([128, 4], I32)
                nc.vector.tensor_scalar(out=ii, in0=veps.bitcast(I32),
                                        scalar1=1, scalar2=-1,
                                        op0=ALU.arith_shift_right,
                                        op1=ALU.bitwise_xor)
                r0 = act.tile([128, 4], F32)
                nc.vector.tensor_scalar(out=r0.bitcast(I32), in0=ii,
                                        scalar1=0x5f3759df + 1, scalar2=None,
                                        op0=ALU.add)
                tN = act.tile([128, 4], F32)
                nc.gpsimd.tensor_tensor(out=tN, in0=r0, in1=r0, op=ALU.mult)
                nc.gpsimd.tensor_tensor(out=tN, in0=tN, in1=veps,
                                        op=ALU.mult)
                nc.vector.tensor_scalar(out=tN, in0=tN, scalar1=-0.5,
                                        scalar2=1.5, op0=ALU.mult,
                                        op1=ALU.add)
                r1 = act.tile([128, 4], F32)
                nc.gpsimd.tensor_tensor(out=r1, in0=r0, in1=tN, op=ALU.mult)
                # second Newton iteration (x4 folded into the affine)
                tM = act.tile([128, 4], F32)
                nc.gpsimd.tensor_tensor(out=tM, in0=r1, in1=r1, op=ALU.mult)
                nc.gpsimd.tensor_tensor(out=tM, in0=tM, in1=veps,
                                        op=ALU.mult)
                nc.vector.tensor_scalar(out=tM, in0=tM, scalar1=-2.0,
                                        scalar2=6.0, op0=ALU.mult,
                                        op1=ALU.add)
                rstd4 = act.tile([128, 4], F32)
                nc.gpsimd.tensor_tensor(out=rstd4, in0=r1, in1=tM,
                                        op=ALU.mult)

                # gs = tanh*(0.5*rstd) + 0.5*rstd, via stride-0 broadcast of
                # rstd4 (== 0.5*rstd_true) along k; Pool tensor_tensor ops
                Rb = _ap_view(rstd4, 0, [[1, 4], [0, 64]])
                Mb = _ap_view(m, 0, [[1, 4], [0, 64]])
                gs = act.tile([128, 4, 64], F16)
                nc.gpsimd.tensor_tensor(out=gs, in0=tcS[:, 4:8, :], in1=Rb,
                                        op=ALU.mult)
                nc.gpsimd.tensor_tensor(out=gs, in0=gs, in1=Rb, op=ALU.add)
                olN = act.tile([128, 4, 64], F16)
                nc.gpsimd.tensor_tensor(out=olN, in0=tcS[:, 0:4, :], in1=Mb,
                                        op=ALU.subtract)
                st["gs"], st["olN"] = gs, olN

            def tail_out(st, t):
                """outF + store, three tiles late."""
                outF = act.tile([128, 4, K], F16)
                nc.gpsimd.tensor_tensor(out=outF, in0=st["olN"],
                                        in1=st["gs"], op=ALU.mult)
                nc.sync.dma_start(out=out[t], in_=outF)

            sts = [{"eq_sb": eq0, "inv_sb": inv0},
                   {"eq_sb": eq1, "inv_sb": inv1}]
            for t in range(NT + 4):
                # prefetch first: the out-DMA below blocks the SP queue on
                # outF, and inputs must stay well ahead of the xy matmuls
                if 2 <= t + 2 < NT:
                    sts.append(prefetch(t + 2))
                if 0 <= t - 3 < NT:
                    trans(sts[t - 3])
                    tail_stats(sts[t - 3])
                if 0 <= t - 2 < NT:
                    head2(sts[t - 2])
                if t - 4 >= 0:
                    tail_out(sts[t - 4], t - 4)
                if t < NT:
                    head(t, sts[t])
    nc.compile()
    return nc


def _prep_weights(mean_inv, std_inv, rms_gamma, W0, W1, W2, w_cross, w_dot,
                  g_w1, g_b1, g_w2, g_b2):
    import ml_dtypes
    f8 = ml_dtypes.float8_e4m3fn
    g = (rms_gamma.astype(np.float64) / np.sqrt(M))
    W0s = W0.astype(np.float64) * g[:, None]
    Wy1 = (W1.astype(np.float64) * g[:, None]) @ (
        w_cross.T.astype(np.float64) / np.sqrt(2.0 * K))
    Wy2 = (W2.astype(np.float64) * g[:, None]) @ (
        w_dot.T.astype(np.float64) / np.sqrt(3.0 * K))
    Wcat = np.concatenate([W0s, Wy1, Wy2], axis=1)          # [256, 192]
    wcat = Wcat.reshape(2, 128, 192).transpose(1, 0, 2).astype(np.float16)

    inv_std = 1.0 / std_inv.astype(np.float64)
    GW1 = g_w1.astype(np.float64) * inv_std[:, None] * GW_SCALE  # [256, 512]
    w1r = GW1.reshape(2, 128, 4, 128).transpose(1, 0, 2, 3)
    w1hi = w1r.astype(f8)
    w1lo = (w1r - w1hi.astype(np.float64)).astype(f8)
    gw1 = np.stack([w1hi, w1lo], axis=1)       # [128, hl, kt, g, h]
    GB1 = g_b1.astype(np.float64) - (mean_inv.astype(np.float64)
                                     * inv_std) @ g_w1.astype(np.float64)
    gb1 = GB1.reshape(4, 128).T.astype(np.float32).copy()
    gw2 = g_w2.astype(np.float64).reshape(4, 128, K).transpose(
        1, 0, 2).astype(np.float16)            # [128, hh, k]
    gb2 = (0.5 * g_b2.astype(np.float64)).reshape(K, 1).astype(np.float32)
    idm = np.zeros((128, 136), np.float16)
    idm[:, :128] = np.eye(128, dtype=np.float16)
    idm[0:64, 128] = 1.0          # S1 mask for hf=0
    idm[64:128, 129] = 1.0        # S1 mask for hf=1
    idm[:, 130:134] = 64.0 * LN_EPS
    return dict(wcat=np.ascontiguousarray(wcat), gw1=np.ascontiguousarray(gw1),
                gb1=gb1, gw2=np.ascontiguousarray(gw2), gb2=gb2, idm=idm)


def _prep_acts(emb):
    """emb [N, 1024] -> per-core (eqT fp16, invT fp8) feature-major."""
    import ml_dtypes
    f8 = ml_dtypes.float8_e4m3fn
    res = []
    for c in range(N_CORES):
        A = np.asarray(emb[c * N_CORE:(c + 1) * N_CORE])
        E = A[:, INV:].reshape(NT, T, 2, 128, 3).transpose(3, 0, 4, 2, 1)
        eqT = np.ascontiguousarray(E.reshape(128, NT, 6, T).astype(np.float16))
        I = A[:, :INV].reshape(NT, T, 2, 128).transpose(3, 0, 2, 1)
        ihi = I.astype(f8)
        ilo = (I - ihi.astype(np.float64)).astype(f8)
        invT = np.ascontiguousarray(
            np.stack([ihi, ilo], axis=2))      # [128, t, hl, hh, a]
        res.append((eqT, invT))
    return res


def kernel(atomic_embeddings, mean_inv, std_inv, rms_gamma, W0, W1, W2,
           w_cross, w_dot, ln_w, ln_b, g_w1, g_b1, g_w2, g_b2):
    global _NC_CACHE, LAST_RESULT
    assert np.allclose(np.asarray(ln_w), 1.0) and np.allclose(np.asarray(ln_b), 0.0), \
        "kernel specialized for ln_w=1, ln_b=0"
    weights = _prep_weights(np.asarray(mean_inv), np.asarray(std_inv),
                            np.asarray(rms_gamma), np.asarray(W0),
                            np.asarray(W1), np.asarray(W2),
                            np.asarray(w_cross), np.asarray(w_dot),
                            np.asarray(g_w1), np.asarray(g_b1),
                            np.asarray(g_w2), np.asarray(g_b2))
    acts = _prep_acts(np.asarray(atomic_embeddings))
    if _NC_CACHE is None:
        _NC_CACHE = build_nc()
    nc = _NC_CACHE
    in_maps = []
    for c in range(N_CORES):
        mm = dict(weights)
        mm["eqT"], mm["invT"] = acts[c]
        in_maps.append(mm)
    trace = bool(int(os.environ.get("CHIRAL_TRACE", "0")))
    try:
        from antenv import axon_hooks  # noqa: F401
    except ImportError:
        os.environ["BASS_NEVER_TRACE"] = "1"
        trace = False
    res = run_bass_kernel_spmd(nc, in_maps, core_ids=list(range(N_CORES)),
                               trace=trace)
    LAST_RESULT = res
    outs = []
    for c in range(N_CORES):
        o = res.results[c]["out"]                    # [NT, 128, 4, 64] fp16
        o = o.reshape(NT, 128, 2, 2, K).transpose(0, 3, 2, 1, 4)
        outs.append(o.reshape(N_CORE, K).astype(np.float32))
    return np.concatenate(outs, axis=0)


# revision 4
# speedup vs baseline: 1.0072x; 1.0039x over previous
"""Trainium2 Bass kernel for nn_ChiralEmbeddingModel — v2.

vs v1 baseline:
 - host pre-transposes activations to feature-major HBM layout: no on-chip
   input transposes
 - gate path (inv @ g_w1 -> silu -> @ g_w2 -> sigmoid) in fp8e4 DoubleRow
   matmuls (0.5 cy/row, 2 k-tiles packed: 4x vs fp16)
 - eq path fp16; cross/dot chain on DVE (one PSUM operand per op, x0
   staged to SBUF via ACT)
 - chi/tanh transposed to atom-major via regular matmuls against
   [identity | hf-masks]: transpose AND LayerNorm K-sums in one pass
 - Newton rsqrt + LN/gate scale fixups on GPSIMD; outF on GPSIMD
 - 4-deep software pipelining so every cross-engine/psum dependency has
   at least one full tile-iteration of slack
 - outputs fp16 in (t, p, s, k) layout; host reorders + casts to fp32
"""
import os
import sys

sys.path.insert(0, '/opt/trn_rl_repo')

import numpy as np

import concourse.bass as bass
import concourse.bacc as bacc
import concourse.mybir as mybir
import concourse.tile as tile
from concourse.bass_utils import run_bass_kernel_spmd

N, INV, M, K, H = 131072, 256, 256, 64, 512
N_CORES = 8
N_CORE = N // N_CORES          # 16384 atoms per core
T = 512                        # atoms per tile
Th = T // 2
NT = N_CORE // T               # 32 tiles
LN_EPS = 1e-5
GW_SCALE = 16.0                # gate weights stored x16 in fp8; undone in ACT
F16 = mybir.dt.float16
F32 = mybir.dt.float32
F8 = mybir.dt.float8e4
I32 = mybir.dt.int32
AF = mybir.ActivationFunctionType
ALU = mybir.AluOpType
DR = mybir.MatmulPerfMode.DoubleRow

LAST_RESULT = None
_NC_CACHE = None


def _ap_view(t, offset_elems, dims):
    """Raw AP on tile t's tensor: partition dim kept, custom free dims."""
    return bass.AP(tensor=t.tensor, offset=t.offset + offset_elems,
                   ap=[list(t.ap[0])] + [list(d) for d in dims])


def build_nc():
    nc = bacc.Bacc("TRN2", target_bir_lowering=False)
    eqT = nc.dram_tensor("eqT", [128, NT, 12, Th], F16, kind="ExternalInput")
    invT = nc.dram_tensor("invT", [128, NT, 2, 2, T], F8, kind="ExternalInput")
    wcat = nc.dram_tensor("wcat", [128, 2, 192], F16, kind="ExternalInput")
    gw1 = nc.dram_tensor("gw1", [128, 2, 2, 4, 128], F8, kind="ExternalInput")
    gb1 = nc.dram_tensor("gb1", [128, 4], F32, kind="ExternalInput")
    gw2 = nc.dram_tensor("gw2", [128, 4, K], F16, kind="ExternalInput")
    gb2 = nc.dram_tensor("gb2", [64, 1], F32, kind="ExternalInput")
    idm = nc.dram_tensor("idm", [128, 136], F16, kind="ExternalInput")
    out = nc.dram_tensor("out", [NT, 128, 4, K], F16, kind="ExternalOutput")

    with tile.TileContext(nc) as tc:
        with (
            tc.tile_pool(name="const", bufs=1) as const,
            tc.tile_pool(name="inp", bufs=7) as inp,
            tc.tile_pool(name="act", bufs=5) as act,
            tc.tile_pool(name="ps", bufs=1, space="PSUM") as ps,
        ):
            # startup DMA order = first-use order: gate-1 inputs first so
            # the PE starts ~3us in instead of ~10us (SP queue + shared
            # HWDGE serialize these)
            inv0 = const.tile([128, 2, 2, T], F8)
            nc.sync.dma_start(out=inv0, in_=invT[:, 0])
            gw1_sb = const.tile([128, 2, 2, 4, 128], F8)
            nc.sync.dma_start(out=gw1_sb, in_=gw1[:, :, :, :, :])
            gb1_sb = const.tile([128, 4], F32)
            nc.sync.dma_start(out=gb1_sb, in_=gb1[:, :])
            eq0 = const.tile([128, 12, Th], F16)
            nc.sync.dma_start(out=eq0, in_=eqT[:, 0])
            wcat_sb = const.tile([128, 2, 192], F16)
            nc.sync.dma_start(out=wcat_sb, in_=wcat[:, :, :])
            inv1 = const.tile([128, 2, 2, T], F8)
            nc.sync.dma_start(out=inv1, in_=invT[:, 1])
            eq1 = const.tile([128, 12, Th], F16)
            nc.sync.dma_start(out=eq1, in_=eqT[:, 1])
            gw2_sb = const.tile([128, 4, K], F16)
            nc.sync.dma_start(out=gw2_sb, in_=gw2[:, :, :])
            gb2_sb = const.tile([64, 1], F32)
            nc.sync.dma_start(out=gb2_sb, in_=gb2[:, :])
            idm_sb = const.tile([128, 136], F16)
            nc.sync.dma_start(out=idm_sb, in_=idm[:, :])
            # sq-with-eps tile: col 64 holds 64*eps; cols 0:64 rewritten
            sqe = const.tile([128, 4, 65], F16)
            nc.sync.dma_start(
                out=_ap_view(sqe, 64, [[65, 4], [1, 1]]),
                in_=idm[:, 130:134].rearrange("p (f x) -> p f x", x=1))

            def prefetch(t):
                eq_sb = inp.tile([128, 12, Th], F16)
                nc.sync.dma_start(out=eq_sb, in_=eqT[:, t])
                inv_sb = inp.tile([128, 2, 2, T], F8)
                nc.sync.dma_start(out=inv_sb, in_=invT[:, t])
                return {"eq_sb": eq_sb, "inv_sb": inv_sb}

            def xmm_m(eq_sb, dst, w, sl2, sl1):
                # per (hf, hh): one 512-col matmul covers c0,c1 (bank-
                # aligned dst slots sl2:sl2+2) + one 256-col for c2 (sl1)
                for hf in range(2):
                    for hh in range(2):
                        b = 6 * hf + 3 * hh
                        nc.tensor.matmul(
                            dst[64 * hf:64 * (hf + 1), sl2:sl2 + 2, :],
                            wcat_sb[:, hh, 64 * w:64 * (w + 1)],
                            eq_sb[:, b:b + 2, :],
                            start=(hh == 0), stop=(hh == 1),
                            tile_position=(0, 64 * hf))
                        nc.tensor.matmul(
                            dst[64 * hf:64 * (hf + 1), sl1, :],
                            wcat_sb[:, hh, 64 * w:64 * (w + 1)],
                            eq_sb[:, b + 2, :],
                            start=(hh == 0), stop=(hh == 1),
                            tile_position=(0, 64 * hf))

            def head(t, st):
                eq_sb, inv_sb = st["eq_sb"], st["inv_sb"]

                # gate layer 1: residual-compensated fp8 DR
                # z1 ~= xhi@whi + xhi@wlo + xlo@whi  (x = inv, w = gw1*16)
                g1s = act.tile([128, 4, T], F16)
                st["g1s"] = g1s
                for g in range(4):
                    g1p = ps.tile([128, T], F32, tag="g1", bufs=2)
                    for i, (hl, xl) in enumerate(((0, 0), (1, 0), (0, 1))):
                        nc.tensor.matmul(g1p, gw1_sb[:, hl, :, g, :],
                                         inv_sb[:, xl, :, :],
                                         start=(i == 0), stop=(i == 2),
                                         perf_mode=DR)
                    nc.scalar.activation(out=g1s[:, g, :], in_=g1p,
                                         func=AF.Silu, scale=1.0 / GW_SCALE,
                                         bias=gb1_sb[:, g:g + 1])

                # eq GEMMs: x0/y1 share one psum tile (slots 0:3 / 3:6);
                # ACT stages the whole tile to SBUF fp16 (hw allows only one
                # PSUM operand per vector op; SBUF fp16 also gives 2x DVE)
                # xy slots: x0 at 0,1,4; y1 at 2,3,5 (c-order per mat)
                xy = ps.tile([128, 6, Th], F32, tag="xy")
                xyS = act.tile([128, 6, Th], F16)
                xmm_m(eq_sb, xy, 0, 0, 4)
                xmm_m(eq_sb, xy, 1, 2, 5)
                nc.scalar.copy(out=xyS, in_=xy)

                # cross products:
                # P0=x0_1*y1_2, P1=x0_2*y1_1 | P2=x0_2*y1_0, P3=x0_0*y1_2
                # P4=x0_0*y1_1, P5=x0_1*y1_0
                Pall = act.tile([128, 6, Th], F16)
                # P0=x0_1*y1_2: slots (1,4)x(5,3); P1=x0_2*y1_1
                nc.vector.tensor_tensor(
                    out=Pall[:, 0:2, :],
                    in0=_ap_view(xyS, 1 * Th, [[3 * Th, 2], [1, Th]]),
                    in1=_ap_view(xyS, 5 * Th, [[-2 * Th, 2], [1, Th]]),
                    op=ALU.mult)
                # P2=x0_2*y1_0: slots (4,0)x(2,5); P3=x0_0*y1_2
                nc.vector.tensor_tensor(
                    out=Pall[:, 2:4, :],
                    in0=_ap_view(xyS, 4 * Th, [[-4 * Th, 2], [1, Th]]),
                    in1=_ap_view(xyS, 2 * Th, [[3 * Th, 2], [1, Th]]),
                    op=ALU.mult)
                # P4=x0_0*y1_1: slots (0,1)x(3,2); P5=x0_1*y1_0
                nc.vector.tensor_tensor(
                    out=Pall[:, 4:6, :],
                    in0=_ap_view(xyS, 0, [[Th, 2], [1, Th]]),
                    in1=_ap_view(xyS, 3 * Th, [[-Th, 2], [1, Th]]),
                    op=ALU.mult)
                crossall = act.tile([128, 3, Th], F16)
                pv = Pall.rearrange("p (c two) f -> p c two f", two=2)
                nc.vector.tensor_tensor(out=crossall, in0=pv[:, :, 0, :],
                                        in1=pv[:, :, 1, :], op=ALU.subtract)
                st["crossall"] = crossall

            def head2(st):
                """y2 GEMM + dot + gate layer 2, one tile late: psum reuse
                and the pd chain get a full iteration of slack."""
                eq_sb, crossall = st["eq_sb"], st["crossall"]
                y2p = ps.tile([128, 3, Th], F32, tag="y2")
                xmm_m(eq_sb, y2p, 2, 0, 2)

                # dot: chi = sum_c cross_c * y2_c  (y2 read from PSUM)
                pd = act.tile([128, 3, Th], F16)
                nc.vector.tensor_tensor(out=pd, in0=crossall, in1=y2p,
                                        op=ALU.mult)
                chiF = act.tile([128, Th], F16)
                nc.vector.tensor_tensor(out=chiF, in0=pd[:, 0, :],
                                        in1=pd[:, 1, :], op=ALU.add)
                nc.vector.tensor_tensor(out=chiF, in0=chiF, in1=pd[:, 2, :],
                                        op=ALU.add)
                st["chiF"] = chiF

                # gate layer 2 (fp16, accumulate over 4 H-blocks) + tanh
                g2p = ps.tile([64, T], F32, tag="gt")
                for j in range(4):
                    nc.tensor.matmul(g2p, gw2_sb[:, j, :],
                                     st["g1s"][:, j, :],
                                     start=(j == 0), stop=(j == 3))
                tanhS = act.tile([64, T], F16)
                nc.scalar.activation(out=tanhS, in_=g2p, func=AF.Tanh,
                                     scale=0.5, bias=gb2_sb[:, 0:1])
                st["tanhS"] = tanhS

            def trans(st):
                """chi/tanh -> atom-major fp16 psum (own 1-bank tag).
                tp slots: [0:4) chi as [r, hf, k]; [4:8) tanh blocks q."""
                chiF, tanhS = st["chiF"], st["tanhS"]
                tp = ps.tile([128, 8, 64], F16, tag="gt")
                for r in range(2):
                    nc.tensor.transpose(
                        tp[:, 2 * r:2 * r + 2, :],
                        chiF[:, r * 128:(r + 1) * 128], idm_sb[:, 0:128])
                for q in range(4):
                    s = (q % 2) * 2 + q // 2
                    nc.tensor.transpose(
                        tp[:, 4 + s, :],
                        tanhS[:, q * 128:(q + 1) * 128],
                        idm_sb[0:64, 0:64])
                st["tp"] = tp

            def tail_stats(st):
                """LN stats + mean/rstd/gate-scale tail (DVE + GPSIMD)."""
                tp = st["tp"]
                # stage to SBUF (psum one-input rule for sq; cheap 2x copy)
                tcS = act.tile([128, 8, 64], F16)
                nc.vector.tensor_copy(out=tcS, in_=tp)
                chg = tcS[:, 0:4, :]                   # [128, s=(r,hf), k]
                S1 = act.tile([128, 4], F32)
                nc.vector.tensor_reduce(out=S1, in_=chg,
                                        axis=mybir.AxisListType.X, op=ALU.add)
                nc.vector.tensor_tensor(out=sqe[:, :, 0:64], in0=chg,
                                        in1=chg, op=ALU.mult)
                S2 = act.tile([128, 4], F32)
                nc.vector.tensor_reduce(out=S2, in_=sqe,
                                        axis=mybir.AxisListType.X, op=ALU.add)

                # veps = S2 - S1^2/64 (+64eps already inside S2)
                t1 = act.tile([128, 4], F32)
                nc.gpsimd.tensor_tensor(out=t1, in0=S1, in1=S1, op=ALU.mult)
                veps = act.tile([128, 4], F32)
                nc.vector.scalar_tensor_tensor(out=veps, in0=t1,
                                               scalar=-1.0 / 64.0, in1=S2,
                                               op0=ALU.mult, op1=ALU.add)
                m = act.tile([128, 4], F32)
                nc.vector.tensor_scalar(out=m, in0=S1, scalar1=1.0 / 64.0,
                                        scalar2=None, op0=ALU.mult)
                # Newton rsqrt (1 iter), x4 folded: rstd4 = 4/sqrt(veps)
                # (true rstd = 8/sqrt(veps) = 2*rstd4)
                ii = act.tile([128, 4], I32)
                nc.vector.tensor_scalar(out=ii, in0=veps.bitcast(I32),
                                        scalar1=1, scalar2=-1,
                                        op0=ALU.arith_shift_right,
                                        op1=ALU.bitwise_xor)
                r0 = act.tile([128, 4], F32)
                nc.vector.tensor_scalar(out=r0.bitcast(I32), in0=ii,
                                        scalar1=0x5f3759df + 1, scalar2=None,
                                        op0=ALU.add)
                tN = act.tile([128, 4], F32)
                nc.gpsimd.tensor_tensor(out=tN, in0=r0, in1=r0, op=ALU.mult)
                nc.gpsimd.tensor_tensor(out=tN, in0=tN, in1=veps,
                                        op=ALU.mult)
                nc.vector.tensor_scalar(out=tN, in0=tN, scalar1=-0.5,
                                        scalar2=1.5, op0=ALU.mult,
                                        op1=ALU.add)
                r1 = act.tile([128, 4], F32)
                nc.gpsimd.tensor_tensor(out=r1, in0=r0, in1=tN, op=ALU.mult)
                # second Newton iteration (x4 folded into the affine)
                tM = act.tile([128, 4], F32)
                nc.gpsimd.tensor_tensor(out=tM, in0=r1, in1=r1, op=ALU.mult)
                nc.gpsimd.tensor_tensor(out=tM, in0=tM, in1=veps,
                                        op=ALU.mult)
                nc.vector.tensor_scalar(out=tM, in0=tM, scalar1=-2.0,
                                        scalar2=6.0, op0=ALU.mult,
                                        op1=ALU.add)
                rstd4 = act.tile([128, 4], F32)
                nc.gpsimd.tensor_tensor(out=rstd4, in0=r1, in1=tM,
                                        op=ALU.mult)

                # gs = tanh*(0.5*rstd) + 0.5*rstd, via stride-0 broadcast of
                # rstd4 (== 0.5*rstd_true) along k; Pool tensor_tensor ops
                Rb = _ap_view(rstd4, 0, [[1, 4], [0, 64]])
                Mb = _ap_view(m, 0, [[1, 4], [0, 64]])
                gs = act.tile([128, 4, 64], F16)
                nc.gpsimd.tensor_tensor(out=gs, in0=tcS[:, 4:8, :], in1=Rb,
                                        op=ALU.mult)
                nc.gpsimd.tensor_tensor(out=gs, in0=gs, in1=Rb, op=ALU.add)
                olN = act.tile([128, 4, 64], F16)
                nc.gpsimd.tensor_tensor(out=olN, in0=tcS[:, 0:4, :], in1=Mb,
                                        op=ALU.subtract)
                st["gs"], st["olN"] = gs, olN

            def tail_out(st, t):
                """outF + store, three tiles late."""
                outF = act.tile([128, 4, K], F16)
                nc.gpsimd.tensor_tensor(out=outF, in0=st["olN"],
                                        in1=st["gs"], op=ALU.mult)
                nc.sync.dma_start(out=out[t], in_=outF)

            sts = [{"eq_sb": eq0, "inv_sb": inv0},
                   {"eq_sb": eq1, "inv_sb": inv1}]
            for t in range(NT + 4):
                # prefetch first: the out-DMA below blocks the SP queue on
                # outF, and inputs must stay well ahead of the xy matmuls
                if 2 <= t + 2 < NT:
                    sts.append(prefetch(t + 2))
                if 0 <= t - 3 < NT:
                    trans(sts[t - 3])
                    tail_stats(sts[t - 3])
                if 0 <= t - 2 < NT:
                    head2(sts[t - 2])
                if t - 4 >= 0:
                    tail_out(sts[t - 4], t - 4)
                if t < NT:
                    head(t, sts[t])
    nc.compile()
    return nc


def _prep_weights(mean_inv, std_inv, rms_gamma, W0, W1, W2, w_cross, w_dot,
                  g_w1, g_b1, g_w2, g_b2):
    import ml_dtypes
    f8 = ml_dtypes.float8_e4m3fn
    g = (rms_gamma.astype(np.float64) / np.sqrt(M))
    W0s = W0.astype(np.float64) * g[:, None]
    Wy1 = (W1.astype(np.float64) * g[:, None]) @ (
        w_cross.T.astype(np.float64) / np.sqrt(2.0 * K))
    Wy2 = (W2.astype(np.float64) * g[:, None]) @ (
        w_dot.T.astype(np.float64) / np.sqrt(3.0 * K))
    Wcat = np.concatenate([W0s, Wy1, Wy2], axis=1)          # [256, 192]
    wcat = Wcat.reshape(2, 128, 192).transpose(1, 0, 2).astype(np.float16)

    inv_std = 1.0 / std_inv.astype(np.float64)
    GW1 = g_w1.astype(np.float64) * inv_std[:, None] * GW_SCALE  # [256, 512]
    w1r = GW1.reshape(2, 128, 4, 128).transpose(1, 0, 2, 3)
    w1hi = w1r.astype(f8)
    w1lo = (w1r - w1hi.astype(np.float64)).astype(f8)
    gw1 = np.stack([w1hi, w1lo], axis=1)       # [128, hl, kt, g, h]
    GB1 = g_b1.astype(np.float64) - (mean_inv.astype(np.float64)
                                     * inv_std) @ g_w1.astype(np.float64)
    gb1 = GB1.reshape(4, 128).T.astype(np.float32).copy()
    gw2 = g_w2.astype(np.float64).reshape(4, 128, K).transpose(
        1, 0, 2).astype(np.float16)            # [128, hh, k]
    gb2 = (0.5 * g_b2.astype(np.float64)).reshape(K, 1).astype(np.float32)
    idm = np.zeros((128, 136), np.float16)
    idm[:, :128] = np.eye(128, dtype=np.float16)
    idm[0:64, 128] = 1.0          # S1 mask for hf=0
    idm[64:128, 129] = 1.0        # S1 mask for hf=1
    idm[:, 130:134] = 64.0 * LN_EPS
    return dict(wcat=np.ascontiguousarray(wcat), gw1=np.ascontiguousarray(gw1),
                gb1=gb1, gw2=np.ascontiguousarray(gw2), gb2=gb2, idm=idm)


def _prep_acts(emb):
    """emb [N, 1024] -> per-core (eqT fp16, invT fp8) feature-major."""
    import ml_dtypes
    f8 = ml_dtypes.float8_e4m3fn
    res = []
    for c in range(N_CORES):
        A = np.asarray(emb[c * N_CORE:(c + 1) * N_CORE])
        E = A[:, INV:].reshape(NT, 2, Th, 2, 128, 3)   # [t, hf, a', hh, p, c]
        E = E.transpose(4, 0, 1, 3, 5, 2)              # [p, t, hf, hh, c, a']
        eqT = np.ascontiguousarray(
            E.reshape(128, NT, 12, Th).astype(np.float16))
        I = A[:, :INV].reshape(NT, T, 2, 128).transpose(3, 0, 2, 1)
        ihi = I.astype(f8)
        ilo = (I - ihi.astype(np.float64)).astype(f8)
        invT = np.ascontiguousarray(
            np.stack([ihi, ilo], axis=2))      # [128, t, hl, hh, a]
        res.append((eqT, invT))
    return res


def kernel(atomic_embeddings, mean_inv, std_inv, rms_gamma, W0, W1, W2,
           w_cross, w_dot, ln_w, ln_b, g_w1, g_b1, g_w2, g_b2):
    global _NC_CACHE, LAST_RESULT
    assert np.allclose(np.asarray(ln_w), 1.0) and np.allclose(np.asarray(ln_b), 0.0), \
        "kernel specialized for ln_w=1, ln_b=0"
    weights = _prep_weights(np.asarray(mean_inv), np.asarray(std_inv),
                            np.asarray(rms_gamma), np.asarray(W0),
                            np.asarray(W1), np.asarray(W2),
                            np.asarray(w_cross), np.asarray(w_dot),
                            np.asarray(g_w1), np.asarray(g_b1),
                            np.asarray(g_w2), np.asarray(g_b2))
    acts = _prep_acts(np.asarray(atomic_embeddings))
    if _NC_CACHE is None:
        _NC_CACHE = build_nc()
    nc = _NC_CACHE
    in_maps = []
    for c in range(N_CORES):
        mm = dict(weights)
        mm["eqT"], mm["invT"] = acts[c]
        in_maps.append(mm)
    trace = bool(int(os.environ.get("CHIRAL_TRACE", "0")))
    try:
        from antenv import axon_hooks  # noqa: F401
    except ImportError:
        os.environ["BASS_NEVER_TRACE"] = "1"
        trace = False
    res = run_bass_kernel_spmd(nc, in_maps, core_ids=list(range(N_CORES)),
                               trace=trace)
    LAST_RESULT = res
    outs = []
    for c in range(N_CORES):
        o = res.results[c]["out"]                    # [NT, 128, 4, 64] fp16
        o = o.reshape(NT, 128, 2, 2, K).transpose(0, 3, 2, 1, 4)
        outs.append(o.reshape(N_CORE, K).astype(np.float32))
    return np.concatenate(outs, axis=0)
